# revision 86
# baseline (speedup 1.0000x reference)
"""Multi-head attention (B=2, S=2048, D=2048, H=16) on 8 Trainium2 cores.

Sharding: core c handles batch b=c//4 and head-group g=c%4 (4 heads, 512
features). All on-device matmuls are fp16; the PE contraction dim is always
the partition dim:

  qT,kT [hd=128, S]  per head   = W[g].T-slices applied to hsT
  v     [S, C=512]
  scoresT[sk, sq]    = kT_h.T @ qT_h          (per head, per 512-col chunk)
  p = exp(scoresT/sqrt(128)) fp16             (no max-subtract: scores O(5))
  po[hd, sq]         = v_h.T @ p              (accumulated over sk blocks)
  l = ones.T @ (sum_sk p); aT = po * (1/l)    (ones-matmul, DVE recip+mul)
  partialT[n, sq]    = wo_g.T @ aT            (per-core partial of o-proj)

Schedule (measured ~381us vs 468us baseline):
- KV phase k0,k1,v0,v1,k2,k3,v2,v3 + q(ch0), one hs chunk load per pass
  (ring bufs=2), weight/hs DMAs chunked+interleaved so the first matmul
  starts ~256KB in; PE warm-up matmuls spin the p-state ramp during the
  cold-start DMA wait.
- Attention supers per sq-chunk ch: PV runs TWO sk behind scores so it
  never waits the Scalar-engine exp stream (~630ns per [128,512] exp vs
  426ns PE per slot); the o-projection matmuls of chunk ch-1 and the
  q-projection matmuls of chunk ch+1 are interleaved as PE filler with
  Bresenham pacing (reserve take()s cover head-boundary dependency tails).
- PSUM budget exactly 8 banks: q-ring 2 + scores-ring 2 + po-ring 2 +
  oproj-ring 2 (KV phase: 4 tags x bufs=2).
Host pre-tiles every DRAM array so all DMAs are contiguous [128, x] blocks;
output partials are written fp16 per-chunk-contiguous and summed/transposed
on host. fp8 DoubleRow was measured (216ns/instr, same as fp16 => no win
after error-compensation) and rejected; pure fp8 fails the 2e-2 gate.
"""
import sys

if "/opt/trn_rl_repo" not in sys.path:
    sys.path.insert(0, "/opt/trn_rl_repo")

import numpy as np

B, S, D, H = 2, 2048, 2048, 16
HD = 128          # head dim
C = 512           # features per core (4 heads)
NB = S // 128     # 16 partition blocks
CH = S // 512     # 4 free-dim chunks
SCALE = 1.0 / np.sqrt(HD)

_BUILT = {}


def _build_program():
    import concourse.bass as bass
    import concourse.tile as tile
    from concourse import bacc, mybir
    from contextlib import ExitStack

    f32 = mybir.dt.float32
    f16 = mybir.dt.float16

    nc = bacc.Bacc("TRN2", target_bir_lowering=False, debug=False, num_devices=1)
    # Host-pre-tiled layouts (see _prep_in_maps):
    #   hs [ch*128+p, d*512+cc]  = hs[b].T[d*128+p, ch*512+cc]
    #   wq/wk/wv [p, d*512+c]    = W[g].T[d*128+p, c]
    #   wo [p, cb*2048+n]        = Wo[:,g].T[cb*128+p, n]
    hsd = nc.dram_tensor("hs", (4 * 128, 16 * 512), f16, kind="ExternalInput").ap()
    wqd = nc.dram_tensor("wq", (128, 16 * 512), f16, kind="ExternalInput").ap()
    wkd = nc.dram_tensor("wk", (128, 16 * 512), f16, kind="ExternalInput").ap()
    wvd = nc.dram_tensor("wv", (128, 16 * 512), f16, kind="ExternalInput").ap()
    wod = nc.dram_tensor("wo", (128, 4 * 2048), f16, kind="ExternalInput").ap()
    # outT layout [p, ch*8192 + nb*512 + c] = partialT[nb*128+p, ch*512+c] so
    # each chunk's output is one contiguous [128, 8192] region (4 group DMAs).
    outd = nc.dram_tensor("outT", (128, 4 * 16 * 512), f16, kind="ExternalOutput").ap()

    with tile.TileContext(nc) as tc, ExitStack() as top:
        dma = nc.gpsimd.dma_start
        Exp = mybir.ActivationFunctionType.Exp

        # ---- persistent SBUF -------------------------------------------
        wp = top.enter_context(tc.tile_pool(name="w", bufs=1))
        wq_sb = wp.tile([128, 8192], f16, name="wq")
        wk_sb = wp.tile([128, 8192], f16, name="wk")
        wv_sb = wp.tile([128, 8192], f16, name="wv")
        wo_sb = wp.tile([128, 8192], f16, name="wo")
        qkp = top.enter_context(tc.tile_pool(name="qk", bufs=1))
        kT = [qkp.tile([128, S], f16, tag=f"kT{h}", name=f"kT{h}") for h in range(4)]
        qtp = top.enter_context(tc.tile_pool(name="qt", bufs=2))
        vp = top.enter_context(tc.tile_pool(name="v", bufs=1))
        v_sb = [vp.tile([128, C], f16, tag=f"v{sb}", name=f"v{sb}") for sb in range(NB)]
        hsp = top.enter_context(tc.tile_pool(name="hs", bufs=2))
        onep = top.enter_context(tc.tile_pool(name="one", bufs=1))
        ones_f = onep.tile([128, 128], f32, name="ones_f")
        nc.vector.memset(ones_f[:], 1.0)
        ones = onep.tile([128, 128], f16, name="ones")
        nc.vector.tensor_copy(ones[:], ones_f[:])

        # PE warm-up: spin the tensor engine on dummy matmuls during the
        # initial DMA wait so the p-state ramp completes before real work.
        wi = onep.tile([128, 512], f16, name="wi")
        nc.vector.memset(wi[:], 1.0)
        with tc.tile_pool(name="warm", bufs=1, space="PSUM") as wps:
            wt = wps.tile([128, 512], f32, name="warm")
            for _ in range(24):
                nc.tensor.matmul(wt[:], lhsT=ones[:], rhs=wi[:],
                                 start=True, stop=True)


        def load_hs(ch, engine=None):
            t = hsp.tile([128, 8192], f16, tag="hs", name=f"hs{ch}")
            d = engine.dma_start if engine is not None else dma
            for i in range(4):
                d(t[:, i * 2048:(i + 1) * 2048],
                  hsd[ch * 128:(ch + 1) * 128, i * 2048:(i + 1) * 2048])
            return t

        # DMA issue order (single gpsimd issue stream, ~640ns per issue):
        # wk+hs0 interleaved first (k0 needs them from t~8us), then hs1
        # (k1, t~21us), then wv (v0, t~40us). Issuing wv before hs1 would
        # add ~4us of issue latency in front of k1's data.
        cuts = [0, 256, 512, 1024, 2048, 4096, 8192]
        for i in range(6):
            dma(wk_sb[:, cuts[i]:cuts[i + 1]], wkd[:, cuts[i]:cuts[i + 1]])
            if i == 0:
                hs_next = hsp.tile([128, 8192], f16, tag="hs", name="hs0")
            dma(hs_next[:, cuts[i]:cuts[i + 1]],
                hsd[0:128, cuts[i]:cuts[i + 1]])
        hs1_t = load_hs(1)
        for i in range(6):
            dma(wv_sb[:, cuts[i]:cuts[i + 1]], wvd[:, cuts[i]:cuts[i + 1]])

        qts = {}
        hs_q = {}

        def q_ops(ch, pool, tagp="q"):
            """Generator: q-projection for chunk ch (64 mm + 4 copies)."""
            hs_t = hs_q[ch]
            qts[ch] = []
            for cb in range(4):
                pq = pool.tile([128, 512], f32, tag=tagp if tagp == "q" else f"a{cb}",
                               name=f"pq{ch}_{cb}")
                for d in range(NB):
                    nc.tensor.matmul(
                        pq[:],
                        lhsT=wq_sb[:, d * 512 + cb * 128:d * 512 + (cb + 1) * 128],
                        rhs=hs_t[:, d * 512:(d + 1) * 512],
                        start=(d == 0), stop=(d == NB - 1))
                    yield
                qt = qtp.tile([128, 512], f16, tag=f"qT{cb}", name=f"qT{ch}_{cb}")
                if pool is psA:
                    nc.scalar.copy(qt[:], pq[:])
                else:
                    nc.vector.tensor_copy(qt[:], pq[:])
                qts[ch].append(qt)
                yield

        # ---- phase KV: kT and v ----------------------------------------
        # Pass order k0,k1,v0,v1,k2,k3,v2,v3: the v-passes (which need wv)
        # start ~35us in, halving the cold-start DMA bandwidth crunch, while
        # the hs ring (bufs=2) still serves every pass exactly once per load.
        def k_pass(ch, hs_t):
            pk = [psA.tile([128, 512], f32, tag=f"a{cb}", name=f"pk{ch}_{cb}")
                  for cb in range(4)]
            for d in range(NB):
                for cb in range(4):
                    nc.tensor.matmul(
                        pk[cb][:],
                        lhsT=wk_sb[:, d * 512 + cb * 128:d * 512 + (cb + 1) * 128],
                        rhs=hs_t[:, d * 512:(d + 1) * 512],
                        start=(d == 0), stop=(d == NB - 1))
            for cb in range(4):
                nc.vector.tensor_copy(kT[cb][:, ch * 512:(ch + 1) * 512], pk[cb][:])

        def v_pass(ch, hs_t):
            pv = [psA.tile([128, 512], f32, tag=f"a{j}", name=f"pv{ch}_{j}")
                  for j in range(4)]
            for d in range(NB):
                for j in range(4):
                    nc.tensor.matmul(
                        pv[j][:],
                        lhsT=hs_t[:, d * 512 + j * 128:d * 512 + (j + 1) * 128],
                        rhs=wv_sb[:, d * 512:(d + 1) * 512],
                        start=(d == 0), stop=(d == NB - 1))
            for j in range(4):
                nc.vector.tensor_copy(v_sb[ch * 4 + j][:], pv[j][:])

        with tc.tile_pool(name="psA", bufs=2, space="PSUM") as psA:
            hs0, hs1 = hs_next, hs1_t
            k_pass(0, hs0)
            k_pass(1, hs1)
            for i in range(4):
                dma(wq_sb[:, i * 2048:(i + 1) * 2048],
                    wqd[:, i * 2048:(i + 1) * 2048])
            for i in range(4):
                dma(wo_sb[:, i * 2048:(i + 1) * 2048],
                    wod[:, i * 2048:(i + 1) * 2048])
            v_pass(0, hs0)
            hs2 = load_hs(2)
            v_pass(1, hs1)
            hs3 = load_hs(3)
            k_pass(2, hs2)
            k_pass(3, hs3)
            v_pass(2, hs2)
            hs_q[0] = load_hs(0)
            v_pass(3, hs3)
            # q(ch0) inside the KV pool: its PSUM banks are already drained,
            # so no bank-transition stall at the phase boundary.
            for _ in q_ops(0, psA, tagp="a"):
                pass
        hs_q[1] = load_hs(1)

        # ---- attention + o-projection supers ---------------------------
        # psS opens first so the scores ring gets banks freed earliest in
        # the KV phase (not q0's freshly-used pq banks) - avoids a stall on
        # super0's first scores matmuls.
        with tc.tile_pool(name="psS", bufs=2, space="PSUM") as psS, \
             tc.tile_pool(name="psQ", bufs=2, space="PSUM") as psQ, \
             tc.tile_pool(name="psP", bufs=2, space="PSUM") as psP, \
             tc.tile_pool(name="psD", bufs=2, space="PSUM") as psD, \
             tc.tile_pool(name="esb", bufs=6) as epool, \
             tc.tile_pool(name="accp", bufs=2) as accp, \
             tc.tile_pool(name="rbp", bufs=2) as rbp, \
             tc.tile_pool(name="aTp", bufs=2) as aTp, \
             tc.tile_pool(name="osb", bufs=2) as opool:

            def d_ops(ch, aTs, on_act=False, pre_pp=None):
                """Generator: o-projection partial for chunk ch. nb-pairs are
                reordered (cb0-2 of both, then cb3+copy of both) so the
                cb3 matmul never waits head3's reciprocal chain. If pre_pp
                holds pre-accumulated (cb0-2) partials for nb0/nb1, only
                their cb3+copy remain and run at the end."""
                ob = opool.tile([128, 8192], f16, tag="ob", name=f"ob{ch}")
                for pr in range(8):
                    if pre_pp is not None and pr == 0:
                        continue
                    pps = []
                    for nb in (2 * pr, 2 * pr + 1):
                        pp = psD.tile([128, 512], f32, tag="pp", name=f"pp{ch}_{nb}")
                        for cb in range(3):
                            nc.tensor.matmul(
                                pp[:],
                                lhsT=wo_sb[:, cb * 2048 + nb * 128:cb * 2048 + (nb + 1) * 128],
                                rhs=aTs[cb][:],
                                start=(cb == 0), stop=False)
                            yield
                        pps.append(pp)
                    for nb, pp in zip((2 * pr, 2 * pr + 1), pps):
                        nc.tensor.matmul(
                            pp[:],
                            lhsT=wo_sb[:, 3 * 2048 + nb * 128:3 * 2048 + (nb + 1) * 128],
                            rhs=aTs[3][:], start=False, stop=True)
                        yield
                        dst = ob[:, nb * 512:(nb + 1) * 512]
                        base = ch * 8192 + nb * 512
                        if on_act and nb >= 12:
                            # late blocks: copy on the scalar engine so the
                            # bank release never queues behind the last
                            # head's reciprocal chain on Vector.
                            nc.scalar.copy(dst, pp[:])
                        else:
                            nc.vector.tensor_copy(dst, pp[:])
                        if ch == 3 and nb >= 14:
                            dma(outd[:, base:base + 256],
                                ob[:, nb * 512:nb * 512 + 256])
                            dma(outd[:, base + 256:base + 512],
                                ob[:, nb * 512 + 256:(nb + 1) * 512])
                        else:
                            dma(outd[:, base:base + 512],
                                ob[:, nb * 512:(nb + 1) * 512])
                        yield
                if pre_pp is not None:
                    for nb, pp in enumerate(pre_pp):
                        nc.tensor.matmul(
                            pp[:],
                            lhsT=wo_sb[:, 3 * 2048 + nb * 128:3 * 2048 + (nb + 1) * 128],
                            rhs=aTs[3][:], start=False, stop=True)
                        yield
                        base = ch * 8192 + nb * 512
                        nc.vector.tensor_copy(ob[:, nb * 512:(nb + 1) * 512], pp[:])
                        dma(outd[:, base:base + 256],
                            ob[:, nb * 512:nb * 512 + 256])
                        dma(outd[:, base + 256:base + 512],
                            ob[:, nb * 512 + 256:(nb + 1) * 512])
                        yield

            def roundrobin(gens):
                """Yield once per underlying op, alternating between gens."""
                gens = list(gens)
                i = 0
                while gens:
                    g = gens[i % len(gens)]
                    try:
                        next(g)
                        i += 1
                        yield
                    except StopIteration:
                        gens.remove(g)

            d_pend = None
            for ch in range(CH):
                if ch < 2:
                    hs_q[ch + 2] = load_hs(ch + 2)
                fill = []
                if d_pend is not None:
                    fill.append(d_pend)
                if ch < 3:
                    fill.append(q_ops(ch + 1, psQ))
                # total filler ops: d_ops yields 80, q_ops yields 68
                nops = (80 if d_pend is not None else 0) + (68 if ch < 3 else 0)
                fillg = roundrobin(fill)
                emitted = 0
                slot = 0
                qT = qts.pop(ch)
                aTs = []
                def pace():
                    nonlocal emitted
                    # Bresenham pacing over 68 virtual slots: keeps a small
                    # reserve for head-boundary dependency tails (take()).
                    want = (nops * slot) // 68
                    while emitted < want:
                        try:
                            next(fillg)
                            emitted += 1
                        except StopIteration:
                            emitted = want
                            break

                def take(k):
                    nonlocal emitted
                    for _ in range(k):
                        try:
                            next(fillg)
                            emitted += 1
                        except StopIteration:
                            break

                pre_pp = None
                pending_pre = []
                for h in range(4):
                    po = psP.tile([128, 512], f32, tag="po", name=f"po{ch}_{h}")
                    acc = accp.tile([128, 512], f16, tag="acc", name=f"acc{ch}_{h}")
                    es = {}
                    # PV runs two sk behind scores so it never waits on exp:
                    # PE slot = [scores(sk), fillers, PV(sk-2)] while ACT
                    # computes exp(sk)/exp(sk-1) in the shadow.
                    for sk in range(NB):
                        ps = psS.tile([128, 512], f32, tag="s", name=f"s{ch}_{h}_{sk}")
                        nc.tensor.matmul(
                            ps[:], lhsT=kT[h][:, sk * 128:(sk + 1) * 128],
                            rhs=qT[h][:],
                            start=True, stop=True)
                        e_t = epool.tile([128, 512], f16, tag="e", name="e_t")
                        nc.scalar.activation(e_t[:], ps[:], Exp, scale=float(SCALE))
                        es[sk] = e_t
                        slot += 1
                        pace()
                        if sk == 3 and pending_pre:
                            # pre-accumulate ch3's first o-proj pair: head
                            # h-1's cb contribution, emitted a few slots in
                            # so its aT-mul has certainly completed.
                            hp, aT_p = pending_pre.pop()
                            for i in range(2):
                                nc.tensor.matmul(
                                    pre_pp[i][:],
                                    lhsT=wo_sb[:, hp * 2048 + i * 128:hp * 2048 + (i + 1) * 128],
                                    rhs=aT_p[:], start=(hp == 0), stop=False)
                        if sk > 1:
                            ep = es.pop(sk - 2)
                            nc.tensor.matmul(
                                po[:], lhsT=v_sb[sk - 2][:, h * 128:(h + 1) * 128],
                                rhs=ep[:], start=(sk == 2), stop=False)
                            if sk == 2:
                                nc.vector.tensor_copy(acc[:], ep[:])
                            else:
                                nc.vector.tensor_add(acc[:], acc[:], ep[:])
                    for skl in (NB - 2, NB - 1):
                        take(2)
                        ep = es.pop(skl)
                        nc.tensor.matmul(
                            po[:], lhsT=v_sb[skl][:, h * 128:(h + 1) * 128],
                            rhs=ep[:], start=False, stop=(skl == NB - 1))
                        nc.vector.tensor_add(acc[:], acc[:], ep[:])
                    take(2)
                    pl = psS.tile([128, 512], f32, tag="s", name=f"pl{ch}_{h}")
                    nc.tensor.matmul(pl[:], lhsT=ones[:], rhs=acc[:],
                                     start=True, stop=True)
                    rb = rbp.tile([128, 512], f32, tag="rb", name=f"rb{ch}_{h}")
                    nc.vector.reciprocal_approx_fast(rb[:], pl[:])
                    aT = aTp.tile([128, 512], f16, tag=f"aT{h}", name=f"aT{ch}_{h}")
                    nc.vector.tensor_mul(aT[:], po[:], rb[:])
                    aTs.append(aT)
                    if ch == 3 and h < 3:
                        if h == 0:
                            pre_pp = [psQ.tile([128, 512], f32, tag="q",
                                               name=f"ppq{i}") for i in range(2)]
                        pending_pre.append((h, aT))
                # drain leftover fillers
                for _ in fillg:
                    pass
                d_pend = d_ops(ch, aTs, on_act=(ch >= 2),
                               pre_pp=pre_pp if ch == 3 else None)
            # final chunk's o-projection
            for _ in d_pend:
                pass

    nc.compile()
    return nc


def _get_program():
    if "nc" not in _BUILT:
        _BUILT["nc"] = _build_program()
    return _BUILT["nc"]


def _prep_in_maps(hs, Wq, Wk, Wv, Wo):
    """Host-side tiling into the DRAM layouts the program expects."""
    in_maps = []
    hs_tiled = []
    for b in range(B):
        t = hs[b].reshape(4, 512, 16, 128).transpose(0, 3, 2, 1).reshape(512, 8192)
        hs_tiled.append(np.ascontiguousarray(t).astype(np.float16))
    for c in range(8):
        b, g = divmod(c, 4)
        sl = slice(g * C, (g + 1) * C)
        wqt = Wq[sl, :].T.reshape(16, 128, 512).transpose(1, 0, 2).reshape(128, 8192)
        wkt = Wk[sl, :].T.reshape(16, 128, 512).transpose(1, 0, 2).reshape(128, 8192)
        wvt = Wv[sl, :].T.reshape(16, 128, 512).transpose(1, 0, 2).reshape(128, 8192)
        wot = Wo[:, sl].T.reshape(4, 128, 2048).transpose(1, 0, 2).reshape(128, 8192)
        in_maps.append({
            "hs": hs_tiled[b],
            "wq": np.ascontiguousarray(wqt).astype(np.float16),
            "wk": np.ascontiguousarray(wkt).astype(np.float16),
            "wv": np.ascontiguousarray(wvt).astype(np.float16),
            "wo": np.ascontiguousarray(wot).astype(np.float16),
        })
    return in_maps


def _reference_fallback(hidden_states, attention_mask, Wq, bq, Wk, bk, Wv, bv, Wo, bo):
    q = hidden_states @ Wq.T + bq
    k = hidden_states @ Wk.T + bk
    v = hidden_states @ Wv.T + bv
    q = q.reshape(B, S, H, HD).transpose(0, 2, 1, 3)
    k = k.reshape(B, S, H, HD).transpose(0, 2, 1, 3)
    v = v.reshape(B, S, H, HD).transpose(0, 2, 1, 3)
    scores = np.einsum("bhqd,bhkd->bhqk", q, k) / np.sqrt(np.float32(HD))
    scores = scores + attention_mask
    scores -= scores.max(axis=-1, keepdims=True)
    e = np.exp(scores)
    attn = e / e.sum(axis=-1, keepdims=True)
    out = np.einsum("bhqk,bhkd->bhqd", attn, v)
    out = out.transpose(0, 2, 1, 3).reshape(B, S, D)
    return (out @ Wo.T + bo).astype(np.float32)


def kernel(hidden_states, attention_mask, Wq, bq, Wk, bk, Wv, bv, Wo, bo):
    from concourse import bass_utils

    hs = np.ascontiguousarray(np.asarray(hidden_states, dtype=np.float32))
    mask = np.asarray(attention_mask, dtype=np.float32)
    Wq = np.asarray(Wq, dtype=np.float32)
    Wk = np.asarray(Wk, dtype=np.float32)
    Wv = np.asarray(Wv, dtype=np.float32)
    Wo = np.asarray(Wo, dtype=np.float32)
    bq = np.asarray(bq, dtype=np.float32)
    bk = np.asarray(bk, dtype=np.float32)
    bv = np.asarray(bv, dtype=np.float32)
    bo = np.asarray(bo, dtype=np.float32)

    # Device program hardcodes zero mask / zero qkv biases (true for this
    # problem's setup_inputs); fall back to exact math if that ever changes.
    if mask.any() or bq.any() or bk.any() or bv.any():
        return _reference_fallback(hs, mask, Wq, bq, Wk, bk, Wv, bv, Wo, bo)

    nc = _get_program()
    in_maps = _prep_in_maps(hs, Wq, Wk, Wv, Wo)
    res = bass_utils.run_bass_kernel_spmd(nc, in_maps, core_ids=list(range(8)))

    out = np.empty((B, S, D), dtype=np.float32)
    for b in range(B):
        acc = res.results[b * 4 + 0]["outT"].astype(np.float32)
        for g in range(1, 4):
            acc = acc + res.results[b * 4 + g]["outT"].astype(np.float32)
        # [p, ch, nb, c] -> partialT[nb*128+p, ch*512+c]; out = partialT.T
        accT = acc.reshape(128, 4, 16, 512).transpose(2, 0, 1, 3).reshape(D, S)
        out[b] = accT.T + bo
    return out


# revision 87
# speedup vs baseline: 1.0204x; 1.0204x over previous
"""Multi-head attention (B=2, S=2048, D=2048, H=16) on 8 Trainium2 cores.

Sharding: core c handles batch b=c//4 and head-group g=c%4 (4 heads, 512
features). All on-device matmuls are fp16; the PE contraction dim is always
the partition dim:

  qT,kT [hd=128, S]  per head   = W[g].T-slices applied to hsT
  v     [S, C=512]
  scoresT[sk, sq]    = kT_h.T @ qT_h          (per head, per 512-col chunk)
  p = exp(scoresT/sqrt(128)) fp16             (no max-subtract: scores O(5))
  po[hd, sq]         = v_h.T @ p              (accumulated over sk blocks)
  l = ones.T @ (sum_sk p); aT = po * (1/l)    (ones-matmul, DVE recip+mul)
  partialT[n, sq]    = wo_g.T @ aT            (per-core partial of o-proj)

Schedule (measured ~381us vs 468us baseline):
- KV phase k0,k1,v0,v1,k2,k3,v2,v3 + q(ch0), one hs chunk load per pass
  (ring bufs=2), weight/hs DMAs chunked+interleaved so the first matmul
  starts ~256KB in; PE warm-up matmuls spin the p-state ramp during the
  cold-start DMA wait.
- Attention supers per sq-chunk ch: PV runs TWO sk behind scores so it
  never waits the Scalar-engine exp stream (~630ns per [128,512] exp vs
  426ns PE per slot); the o-projection matmuls of chunk ch-1 and the
  q-projection matmuls of chunk ch+1 are interleaved as PE filler with
  Bresenham pacing (reserve take()s cover head-boundary dependency tails).
- PSUM budget exactly 8 banks: q-ring 2 + scores-ring 2 + po-ring 2 +
  oproj-ring 2 (KV phase: 4 tags x bufs=2).
Host pre-tiles every DRAM array so all DMAs are contiguous [128, x] blocks;
output partials are written fp16 per-chunk-contiguous and summed/transposed
on host. fp8 DoubleRow was measured (216ns/instr, same as fp16 => no win
after error-compensation) and rejected; pure fp8 fails the 2e-2 gate.
"""
import sys

if "/opt/trn_rl_repo" not in sys.path:
    sys.path.insert(0, "/opt/trn_rl_repo")

import numpy as np

B, S, D, H = 2, 2048, 2048, 16
HD = 128          # head dim
C = 512           # features per core (4 heads)
NB = S // 128     # 16 partition blocks
CH = S // 512     # 4 free-dim chunks
SCALE = 1.0 / np.sqrt(HD)

_BUILT = {}


def _build_program():
    import concourse.bass as bass
    import concourse.tile as tile
    from concourse import bacc, mybir
    from contextlib import ExitStack

    f32 = mybir.dt.float32
    f16 = mybir.dt.float16

    nc = bacc.Bacc("TRN2", target_bir_lowering=False, debug=False, num_devices=1)
    # Host-pre-tiled layouts (see _prep_in_maps):
    #   hs [ch*128+p, d*512+cc]  = hs[b].T[d*128+p, ch*512+cc]
    #   wq/wk/wv [p, d*512+c]    = W[g].T[d*128+p, c]
    #   wo [p, cb*2048+n]        = Wo[:,g].T[cb*128+p, n]
    hsd = nc.dram_tensor("hs", (4 * 128, 16 * 512), f16, kind="ExternalInput").ap()
    wqd = nc.dram_tensor("wq", (128, 16 * 512), f16, kind="ExternalInput").ap()
    wkd = nc.dram_tensor("wk", (128, 16 * 512), f16, kind="ExternalInput").ap()
    wvd = nc.dram_tensor("wv", (128, 16 * 512), f16, kind="ExternalInput").ap()
    wod = nc.dram_tensor("wo", (128, 4 * 2048), f16, kind="ExternalInput").ap()
    # outT layout [p, ch*8192 + nb*512 + c] = partialT[nb*128+p, ch*512+c] so
    # each chunk's output is one contiguous [128, 8192] region (4 group DMAs).
    outd = nc.dram_tensor("outT", (128, 4 * 16 * 512), f16, kind="ExternalOutput").ap()

    with tile.TileContext(nc) as tc, ExitStack() as top:
        dma = nc.gpsimd.dma_start
        Exp = mybir.ActivationFunctionType.Exp

        # ---- persistent SBUF -------------------------------------------
        wp = top.enter_context(tc.tile_pool(name="w", bufs=1))
        wq_sb = wp.tile([128, 8192], f16, name="wq")
        wk_sb = wp.tile([128, 8192], f16, name="wk")
        wv_sb = wp.tile([128, 8192], f16, name="wv")
        wo_sb = wp.tile([128, 8192], f16, name="wo")
        qkp = top.enter_context(tc.tile_pool(name="qk", bufs=1))
        kT = [qkp.tile([128, S], f16, tag=f"kT{h}", name=f"kT{h}") for h in range(4)]
        qtp = top.enter_context(tc.tile_pool(name="qt", bufs=2))
        vp = top.enter_context(tc.tile_pool(name="v", bufs=1))
        v_sb = [vp.tile([128, C], f16, tag=f"v{sb}", name=f"v{sb}") for sb in range(NB)]
        hsp = top.enter_context(tc.tile_pool(name="hs", bufs=2))
        onep = top.enter_context(tc.tile_pool(name="one", bufs=1))
        ones_f = onep.tile([128, 128], f32, name="ones_f")
        nc.vector.memset(ones_f[:], 1.0)
        ones = onep.tile([128, 128], f16, name="ones")
        nc.vector.tensor_copy(ones[:], ones_f[:])

        # PE warm-up: spin the tensor engine on dummy matmuls during the
        # initial DMA wait so the p-state ramp completes before real work.
        wi = onep.tile([128, 512], f16, name="wi")
        nc.vector.memset(wi[:], 1.0)
        with tc.tile_pool(name="warm", bufs=1, space="PSUM") as wps:
            wt = wps.tile([128, 512], f32, name="warm")
            for _ in range(24):
                nc.tensor.matmul(wt[:], lhsT=ones[:], rhs=wi[:],
                                 start=True, stop=True)


        def load_hs(ch, engine=None):
            t = hsp.tile([128, 8192], f16, tag="hs", name=f"hs{ch}")
            d = engine.dma_start if engine is not None else dma
            for i in range(4):
                d(t[:, i * 2048:(i + 1) * 2048],
                  hsd[ch * 128:(ch + 1) * 128, i * 2048:(i + 1) * 2048])
            return t

        # DMA issue order (single gpsimd issue stream, ~640ns per issue):
        # wk+hs0 interleaved first (k0 needs them from t~8us), then hs1
        # (k1, t~21us), then wv (v0, t~40us). Issuing wv before hs1 would
        # add ~4us of issue latency in front of k1's data.
        # 2KB+ lines throughout: the PE warm-ups cover the first ~13us, so
        # tiny early chunks (512B lines, half-rate) would only slow supply.
        cuts = [0, 1024, 2048, 3072, 4096, 6144, 8192]
        for i in range(6):
            dma(wk_sb[:, cuts[i]:cuts[i + 1]], wkd[:, cuts[i]:cuts[i + 1]])
            if i == 0:
                hs_next = hsp.tile([128, 8192], f16, tag="hs", name="hs0")
            dma(hs_next[:, cuts[i]:cuts[i + 1]],
                hsd[0:128, cuts[i]:cuts[i + 1]])
        hs1_t = load_hs(1)
        for i in range(6):
            dma(wv_sb[:, cuts[i]:cuts[i + 1]], wvd[:, cuts[i]:cuts[i + 1]])

        qts = {}
        hs_q = {}

        def q_ops(ch, pool, tagp="q"):
            """Generator: q-projection for chunk ch (64 mm + 4 copies)."""
            hs_t = hs_q[ch]
            qts[ch] = []
            for cb in range(4):
                pq = pool.tile([128, 512], f32, tag=tagp if tagp == "q" else f"a{cb}",
                               name=f"pq{ch}_{cb}")
                for d in range(NB):
                    nc.tensor.matmul(
                        pq[:],
                        lhsT=wq_sb[:, d * 512 + cb * 128:d * 512 + (cb + 1) * 128],
                        rhs=hs_t[:, d * 512:(d + 1) * 512],
                        start=(d == 0), stop=(d == NB - 1))
                    yield
                qt = qtp.tile([128, 512], f16, tag=f"qT{cb}", name=f"qT{ch}_{cb}")
                if pool is psA:
                    nc.scalar.copy(qt[:], pq[:])
                else:
                    nc.vector.tensor_copy(qt[:], pq[:])
                qts[ch].append(qt)
                yield

        # ---- phase KV: kT and v ----------------------------------------
        # Pass order k0,k1,v0,v1,k2,k3,v2,v3: the v-passes (which need wv)
        # start ~35us in, halving the cold-start DMA bandwidth crunch, while
        # the hs ring (bufs=2) still serves every pass exactly once per load.
        def k_pass(ch, hs_t):
            pk = [psA.tile([128, 512], f32, tag=f"a{cb}", name=f"pk{ch}_{cb}")
                  for cb in range(4)]
            for d in range(NB):
                for cb in range(4):
                    nc.tensor.matmul(
                        pk[cb][:],
                        lhsT=wk_sb[:, d * 512 + cb * 128:d * 512 + (cb + 1) * 128],
                        rhs=hs_t[:, d * 512:(d + 1) * 512],
                        start=(d == 0), stop=(d == NB - 1))
            for cb in range(4):
                nc.vector.tensor_copy(kT[cb][:, ch * 512:(ch + 1) * 512], pk[cb][:])

        def v_pass(ch, hs_t):
            pv = [psA.tile([128, 512], f32, tag=f"a{j}", name=f"pv{ch}_{j}")
                  for j in range(4)]
            for d in range(NB):
                for j in range(4):
                    nc.tensor.matmul(
                        pv[j][:],
                        lhsT=hs_t[:, d * 512 + j * 128:d * 512 + (j + 1) * 128],
                        rhs=wv_sb[:, d * 512:(d + 1) * 512],
                        start=(d == 0), stop=(d == NB - 1))
            for j in range(4):
                nc.vector.tensor_copy(v_sb[ch * 4 + j][:], pv[j][:])

        with tc.tile_pool(name="psA", bufs=2, space="PSUM") as psA:
            hs0, hs1 = hs_next, hs1_t
            k_pass(0, hs0)
            k_pass(1, hs1)
            for i in range(4):
                dma(wq_sb[:, i * 2048:(i + 1) * 2048],
                    wqd[:, i * 2048:(i + 1) * 2048])
            for i in range(4):
                dma(wo_sb[:, i * 2048:(i + 1) * 2048],
                    wod[:, i * 2048:(i + 1) * 2048])
            v_pass(0, hs0)
            hs2 = load_hs(2)
            v_pass(1, hs1)
            hs3 = load_hs(3)
            k_pass(2, hs2)
            k_pass(3, hs3)
            v_pass(2, hs2)
            hs_q[0] = load_hs(0)
            v_pass(3, hs3)
            # q(ch0) inside the KV pool: its PSUM banks are already drained,
            # so no bank-transition stall at the phase boundary.
            for _ in q_ops(0, psA, tagp="a"):
                pass
        hs_q[1] = load_hs(1)

        # ---- attention + o-projection supers ---------------------------
        # psS opens first so the scores ring gets banks freed earliest in
        # the KV phase (not q0's freshly-used pq banks) - avoids a stall on
        # super0's first scores matmuls.
        with tc.tile_pool(name="psS", bufs=2, space="PSUM") as psS, \
             tc.tile_pool(name="psQ", bufs=2, space="PSUM") as psQ, \
             tc.tile_pool(name="psP", bufs=2, space="PSUM") as psP, \
             tc.tile_pool(name="psD", bufs=2, space="PSUM") as psD, \
             tc.tile_pool(name="esb", bufs=6) as epool, \
             tc.tile_pool(name="accp", bufs=2) as accp, \
             tc.tile_pool(name="rbp", bufs=2) as rbp, \
             tc.tile_pool(name="aTp", bufs=2) as aTp, \
             tc.tile_pool(name="osb", bufs=2) as opool:

            def d_ops(ch, aTs, on_act=False, pre_pp=None):
                """Generator: o-projection partial for chunk ch. nb-pairs are
                reordered (cb0-2 of both, then cb3+copy of both) so the
                cb3 matmul never waits head3's reciprocal chain. If pre_pp
                holds pre-accumulated (cb0-2) partials for nb0/nb1, only
                their cb3+copy remain and run at the end."""
                ob = opool.tile([128, 8192], f16, tag="ob", name=f"ob{ch}")
                for pr in range(8):
                    if pre_pp is not None and pr == 0:
                        continue
                    pps = []
                    for nb in (2 * pr, 2 * pr + 1):
                        pp = psD.tile([128, 512], f32, tag="pp", name=f"pp{ch}_{nb}")
                        for cb in range(3):
                            nc.tensor.matmul(
                                pp[:],
                                lhsT=wo_sb[:, cb * 2048 + nb * 128:cb * 2048 + (nb + 1) * 128],
                                rhs=aTs[cb][:],
                                start=(cb == 0), stop=False)
                            yield
                        pps.append(pp)
                    for nb, pp in zip((2 * pr, 2 * pr + 1), pps):
                        nc.tensor.matmul(
                            pp[:],
                            lhsT=wo_sb[:, 3 * 2048 + nb * 128:3 * 2048 + (nb + 1) * 128],
                            rhs=aTs[3][:], start=False, stop=True)
                        yield
                        dst = ob[:, nb * 512:(nb + 1) * 512]
                        base = ch * 8192 + nb * 512
                        if on_act and nb >= 12:
                            # late blocks: copy on the scalar engine so the
                            # bank release never queues behind the last
                            # head's reciprocal chain on Vector.
                            nc.scalar.copy(dst, pp[:])
                        else:
                            nc.vector.tensor_copy(dst, pp[:])
                        if ch == 3 and nb >= 14:
                            dma(outd[:, base:base + 256],
                                ob[:, nb * 512:nb * 512 + 256])
                            dma(outd[:, base + 256:base + 512],
                                ob[:, nb * 512 + 256:(nb + 1) * 512])
                        else:
                            dma(outd[:, base:base + 512],
                                ob[:, nb * 512:(nb + 1) * 512])
                        yield
                if pre_pp is not None:
                    for nb, pp in enumerate(pre_pp):
                        nc.tensor.matmul(
                            pp[:],
                            lhsT=wo_sb[:, 3 * 2048 + nb * 128:3 * 2048 + (nb + 1) * 128],
                            rhs=aTs[3][:], start=False, stop=True)
                        yield
                        base = ch * 8192 + nb * 512
                        nc.vector.tensor_copy(ob[:, nb * 512:(nb + 1) * 512], pp[:])
                        dma(outd[:, base:base + 256],
                            ob[:, nb * 512:nb * 512 + 256])
                        dma(outd[:, base + 256:base + 512],
                            ob[:, nb * 512 + 256:(nb + 1) * 512])
                        yield

            def roundrobin(gens):
                """Yield once per underlying op, alternating between gens."""
                gens = list(gens)
                i = 0
                while gens:
                    g = gens[i % len(gens)]
                    try:
                        next(g)
                        i += 1
                        yield
                    except StopIteration:
                        gens.remove(g)

            d_pend = None
            for ch in range(CH):
                if ch < 2:
                    hs_q[ch + 2] = load_hs(ch + 2)
                fill = []
                if d_pend is not None:
                    fill.append(d_pend)
                if ch < 3:
                    fill.append(q_ops(ch + 1, psQ))
                # total filler ops: d_ops yields 80, q_ops yields 68
                nops = (80 if d_pend is not None else 0) + (68 if ch < 3 else 0)
                fillg = roundrobin(fill)
                emitted = 0
                slot = 0
                qT = qts.pop(ch)
                aTs = []
                def pace():
                    nonlocal emitted
                    # Bresenham pacing over 68 virtual slots: keeps a small
                    # reserve for head-boundary dependency tails (take()).
                    want = (nops * slot) // 68
                    while emitted < want:
                        try:
                            next(fillg)
                            emitted += 1
                        except StopIteration:
                            emitted = want
                            break

                def take(k):
                    nonlocal emitted
                    for _ in range(k):
                        try:
                            next(fillg)
                            emitted += 1
                        except StopIteration:
                            break

                pre_pp = None
                pending_pre = []
                for h in range(4):
                    po = psP.tile([128, 512], f32, tag="po", name=f"po{ch}_{h}")
                    acc = accp.tile([128, 512], f16, tag="acc", name=f"acc{ch}_{h}")
                    es = {}
                    # PV runs two sk behind scores so it never waits on exp:
                    # PE slot = [scores(sk), fillers, PV(sk-2)] while ACT
                    # computes exp(sk)/exp(sk-1) in the shadow.
                    for sk in range(NB):
                        ps = psS.tile([128, 512], f32, tag="s", name=f"s{ch}_{h}_{sk}")
                        nc.tensor.matmul(
                            ps[:], lhsT=kT[h][:, sk * 128:(sk + 1) * 128],
                            rhs=qT[h][:],
                            start=True, stop=True)
                        e_t = epool.tile([128, 512], f16, tag="e", name="e_t")
                        nc.scalar.activation(e_t[:], ps[:], Exp, scale=float(SCALE))
                        es[sk] = e_t
                        slot += 1
                        pace()
                        if sk == 3 and pending_pre:
                            # pre-accumulate ch3's first o-proj pair: head
                            # h-1's cb contribution, emitted a few slots in
                            # so its aT-mul has certainly completed.
                            hp, aT_p = pending_pre.pop()
                            for i in range(2):
                                nc.tensor.matmul(
                                    pre_pp[i][:],
                                    lhsT=wo_sb[:, hp * 2048 + i * 128:hp * 2048 + (i + 1) * 128],
                                    rhs=aT_p[:], start=(hp == 0), stop=False)
                        if sk > 1:
                            ep = es.pop(sk - 2)
                            nc.tensor.matmul(
                                po[:], lhsT=v_sb[sk - 2][:, h * 128:(h + 1) * 128],
                                rhs=ep[:], start=(sk == 2), stop=False)
                            if sk == 2:
                                nc.vector.tensor_copy(acc[:], ep[:])
                            else:
                                nc.vector.tensor_add(acc[:], acc[:], ep[:])
                    for skl in (NB - 2, NB - 1):
                        take(2)
                        ep = es.pop(skl)
                        nc.tensor.matmul(
                            po[:], lhsT=v_sb[skl][:, h * 128:(h + 1) * 128],
                            rhs=ep[:], start=False, stop=(skl == NB - 1))
                        nc.vector.tensor_add(acc[:], acc[:], ep[:])
                    take(2)
                    pl = psS.tile([128, 512], f32, tag="s", name=f"pl{ch}_{h}")
                    nc.tensor.matmul(pl[:], lhsT=ones[:], rhs=acc[:],
                                     start=True, stop=True)
                    rb = rbp.tile([128, 512], f32, tag="rb", name=f"rb{ch}_{h}")
                    nc.vector.reciprocal_approx_fast(rb[:], pl[:])
                    aT = aTp.tile([128, 512], f16, tag=f"aT{h}", name=f"aT{ch}_{h}")
                    nc.vector.tensor_mul(aT[:], po[:], rb[:])
                    aTs.append(aT)
                    if ch == 3 and h < 3:
                        if h == 0:
                            pre_pp = [psQ.tile([128, 512], f32, tag="q",
                                               name=f"ppq{i}") for i in range(2)]
                        pending_pre.append((h, aT))
                # drain leftover fillers
                for _ in fillg:
                    pass
                d_pend = d_ops(ch, aTs, on_act=(ch >= 2),
                               pre_pp=pre_pp if ch == 3 else None)
            # final chunk's o-projection
            for _ in d_pend:
                pass

    nc.compile()
    return nc


def _get_program():
    if "nc" not in _BUILT:
        _BUILT["nc"] = _build_program()
    return _BUILT["nc"]


def _prep_in_maps(hs, Wq, Wk, Wv, Wo):
    """Host-side tiling into the DRAM layouts the program expects."""
    in_maps = []
    hs_tiled = []
    for b in range(B):
        t = hs[b].reshape(4, 512, 16, 128).transpose(0, 3, 2, 1).reshape(512, 8192)
        hs_tiled.append(np.ascontiguousarray(t).astype(np.float16))
    for c in range(8):
        b, g = divmod(c, 4)
        sl = slice(g * C, (g + 1) * C)
        wqt = Wq[sl, :].T.reshape(16, 128, 512).transpose(1, 0, 2).reshape(128, 8192)
        wkt = Wk[sl, :].T.reshape(16, 128, 512).transpose(1, 0, 2).reshape(128, 8192)
        wvt = Wv[sl, :].T.reshape(16, 128, 512).transpose(1, 0, 2).reshape(128, 8192)
        wot = Wo[:, sl].T.reshape(4, 128, 2048).transpose(1, 0, 2).reshape(128, 8192)
        in_maps.append({
            "hs": hs_tiled[b],
            "wq": np.ascontiguousarray(wqt).astype(np.float16),
            "wk": np.ascontiguousarray(wkt).astype(np.float16),
            "wv": np.ascontiguousarray(wvt).astype(np.float16),
            "wo": np.ascontiguousarray(wot).astype(np.float16),
        })
    return in_maps


def _reference_fallback(hidden_states, attention_mask, Wq, bq, Wk, bk, Wv, bv, Wo, bo):
    q = hidden_states @ Wq.T + bq
    k = hidden_states @ Wk.T + bk
    v = hidden_states @ Wv.T + bv
    q = q.reshape(B, S, H, HD).transpose(0, 2, 1, 3)
    k = k.reshape(B, S, H, HD).transpose(0, 2, 1, 3)
    v = v.reshape(B, S, H, HD).transpose(0, 2, 1, 3)
    scores = np.einsum("bhqd,bhkd->bhqk", q, k) / np.sqrt(np.float32(HD))
    scores = scores + attention_mask
    scores -= scores.max(axis=-1, keepdims=True)
    e = np.exp(scores)
    attn = e / e.sum(axis=-1, keepdims=True)
    out = np.einsum("bhqk,bhkd->bhqd", attn, v)
    out = out.transpose(0, 2, 1, 3).reshape(B, S, D)
    return (out @ Wo.T + bo).astype(np.float32)


def kernel(hidden_states, attention_mask, Wq, bq, Wk, bk, Wv, bv, Wo, bo):
    from concourse import bass_utils

    hs = np.ascontiguousarray(np.asarray(hidden_states, dtype=np.float32))
    mask = np.asarray(attention_mask, dtype=np.float32)
    Wq = np.asarray(Wq, dtype=np.float32)
    Wk = np.asarray(Wk, dtype=np.float32)
    Wv = np.asarray(Wv, dtype=np.float32)
    Wo = np.asarray(Wo, dtype=np.float32)
    bq = np.asarray(bq, dtype=np.float32)
    bk = np.asarray(bk, dtype=np.float32)
    bv = np.asarray(bv, dtype=np.float32)
    bo = np.asarray(bo, dtype=np.float32)

    # Device program hardcodes zero mask / zero qkv biases (true for this
    # problem's setup_inputs); fall back to exact math if that ever changes.
    if mask.any() or bq.any() or bk.any() or bv.any():
        return _reference_fallback(hs, mask, Wq, bq, Wk, bk, Wv, bv, Wo, bo)

    nc = _get_program()
    in_maps = _prep_in_maps(hs, Wq, Wk, Wv, Wo)
    res = bass_utils.run_bass_kernel_spmd(nc, in_maps, core_ids=list(range(8)))

    out = np.empty((B, S, D), dtype=np.float32)
    for b in range(B):
        acc = res.results[b * 4 + 0]["outT"].astype(np.float32)
        for g in range(1, 4):
            acc = acc + res.results[b * 4 + g]["outT"].astype(np.float32)
        # [p, ch, nb, c] -> partialT[nb*128+p, ch*512+c]; out = partialT.T
        accT = acc.reshape(128, 4, 16, 512).transpose(2, 0, 1, 3).reshape(D, S)
        out[b] = accT.T + bo
    return out
